# revision 39
# baseline (speedup 1.0000x reference)
"""Trainium2 Bass kernel for Attention_concat (separable PAM attention).

Math (per batch b, N = H*W = 4096):
    eqn[n] = wq_eff . x[:, n]                  (wq_eff = Wq^T Wc[:64])
    ekn[m] = wk_eff . x[:, m]
    y[c, m] = x[c, m] + A[c] + Bv[c] * ekn[m]
with global reductions u = x @ 1, t = x @ eqn and
    Bv = g*Wv u + g*N*bv
    A  = g*Wv (t + (bq_eff+bk_eff) u) + bv*(g*E + g*N*(bq_eff+bk_eff))
    E  = wq_eff . u,   g = gamma / N

Precision: the attention correction is ~1.5e-4 of |y|, so the whole pipeline
runs in bf16 (x is loaded bf16, y stored bf16); measured rel-to-scale error
~3e-3 vs the 2e-2 gate.

Sharding: 2 cores per batch, each handles half the spatial columns. Each core
redundantly computes the global reductions over the full x[b] (own half + a
bf16 copy of the other half), then writes its own 2048 output columns.

Engine/DMA split: inputs ride both HWDGE rings (sync: x own half; scalar: the
weight pack first, then x other half) so the eqb-gating weights land early.
PE broadcasts eqn into PSUM (stationary wq_eff replicated along the free dim)
and computes ekn rows; DVE does the t-reduction via scalar_tensor_tensor with
accum_out; ACT accumulates u via activation-Copy accum_out; ekn PSUM->RC
copies split ACT/DVE. The A/Bv rows are assembled in one [2,C] PSUM
accumulation chain (tub stationary + [gN,0]/[0,sc] selector rows against the
bv row) — no cross-partition moves. Phase C: rank-2 AB x RC matmul per
512-block; two blocks finish as DVE adds (x + psum), two as PE identity-fold
plus ACT copy, then 4 output DMAs alternating rings. Dummy matmuls keep the
PE p-state up across idle windows.

Module-level workarounds (this container's walrus accepts only one sync-wait
per instruction): extra waits are hoisted onto single-wait NoOps at BIR level,
and the Tile tail drain is rebuilt the same way.
"""

import json as _json

import numpy as np

import concourse.bass as bass
import concourse.bass2jax as _b2j
import concourse.bass_utils as _bu
import concourse.mybir as mybir
import concourse.tile as tile
from concourse.bass_utils import run_bass_kernel_spmd
from concourse.vector_clock import ScopedClock, VectorClock

B, C, H, W = 4, 256, 64, 64
N = H * W            # 4096
INTER = C // 4       # 64
NCORES = 8
HALF = N // 2        # 2048 output columns per core
F32 = mybir.dt.float32
BF16 = mybir.dt.bfloat16
AX = mybir.AxisListType
OP = mybir.AluOpType
ACTF = mybir.ActivationFunctionType

# wpka free-dim layout (per q chunk): [0]=wq_eff col, [1]=wk_eff col,
# [2:130]=wq_eff replicated 128  (gates phase A -> lands first)
WPKA_COLS = 130
# wpkb: [0:256]=g*Wv^T, [256:384]=identity in q=0  (tail-only -> lands last)
WPKB_COLS = 384
# rpk2 row-pack: [0:256]=bv, [256]=g*N, [257]=0
RPK_COLS = 258


def _split_multi_waits(bir: dict) -> dict:
    """The nix walrus accepts only ONE sync-wait command per instruction.
    Hoist extra waits onto preceding single-wait NoOps on the same engine
    (sequencers execute in program order, so semantics are unchanged)."""
    ctr = 0
    for fn in bir.get("functions", []):
        for blk in fn.get("blocks", []):
            insts = blk.get("instructions")
            if not insts:
                continue
            out = []
            for inst in insts:
                si = inst.get("sync_info") or {}
                waits = si.get("on_wait") or []
                if len(waits) > 1 and inst.get("engine", "Unassigned") != "Unassigned":
                    for w in waits[:-1]:
                        ctr += 1
                        out.append({
                            "debug": inst.get("debug", 0),
                            "engine": inst["engine"],
                            "ins": [], "outs": [],
                            "name": f"{inst['name']}-ws{ctr}",
                            "opcode": "NoOp",
                            "sync_info": {"on_update": [], "on_wait": [w]},
                        })
                    si["on_wait"] = [waits[-1]]
                out.append(inst)
            blk["instructions"] = out
    return bir


_WAIT_SPLIT_DONE = False


def install_wait_split():
    global _WAIT_SPLIT_DONE
    if _WAIT_SPLIT_DONE:
        return
    orig = _bu.compile_bir_kernel

    def wrapped(bir_json, *a, **kw):
        d = _json.loads(bir_json)
        _split_multi_waits(d)
        return orig(_json.dumps(d).encode(), *a, **kw)

    _bu.compile_bir_kernel = wrapped
    _b2j.compile_bir_kernel = wrapped
    _WAIT_SPLIT_DONE = True


class SplitDrainTileContext(tile.TileContext):
    """Tail fix for the same 1-wait walrus limit: park the global-clock waits
    on single-wait Nops spread across all five engines (they wait in
    parallel), then a wait-free drain + the usual barrier/reset."""

    def _drain_and_barrier(self, tick_clock, wait_clock):
        gc = tick_clock.global_clock
        nprocs = len(gc)
        engines = [self.nc.sync, self.nc.vector, self.nc.scalar,
                   self.nc.gpsimd, self.nc.tensor]
        idx = 0
        for proc in range(nprocs):
            if gc[proc] > 0:
                eng = engines[idx % len(engines)]
                idx += 1
                nop = eng.nop(nofuse=True, hint=f"tail_wait_p{proc}")
                vc = VectorClock([0] * nprocs)
                vc.require_at_least(proc, gc[proc])
                wait_clock.add_sem_waits(nop.ins, ScopedClock({None: vc}))
        self.nc.sync.drain()
        self.nc.all_engine_barrier()
        assert self.sems is not None
        popped = self.nc._tile_sem_poison_stack.pop()
        assert popped is self._sem_poison
        self.nc.clear_and_free_semaphores(list(self.sems.allocated().values()))
        self.nc.all_engine_barrier()


def build_kernel(g: float, bq_eff: float, bk_eff: float):
    """Build the per-core Bass program. g = gamma/N."""
    bqk = bq_eff + bk_eff
    nc = bass.Bass()
    xd = [[nc.dram_tensor(f"x{s}{k}", [128, 2, 1024], BF16,
                          kind="ExternalInput")
           for k in range(2)] for s in range(2)]
    wpka = nc.dram_tensor("wpka", [128, 2, WPKA_COLS], BF16, kind="ExternalInput")
    wpkb = nc.dram_tensor("wpkb", [128, 2, WPKB_COLS], BF16, kind="ExternalInput")
    rpk2 = nc.dram_tensor("rpk2", [1, RPK_COLS], BF16, kind="ExternalInput")
    rones = nc.dram_tensor("rones", [1, HALF], BF16, kind="ExternalInput")
    yout = nc.dram_tensor("yout", [128, 2, HALF], BF16, kind="ExternalOutput")

    with SplitDrainTileContext(nc) as tc:
        with (
            tc.tile_pool(name="persist", bufs=1) as pp,
            tc.tile_pool(name="trasha", bufs=1) as tpa,
            tc.tile_pool(name="trashd", bufs=1) as tpd,
            tc.tile_pool(name="ypool", bufs=4) as yp,
            tc.tile_pool(name="psm", bufs=2, space="PSUM") as psm,
            tc.tile_pool(name="pbig", bufs=2, space="PSUM") as pbig,
            tc.tile_pool(name="pwu", bufs=1, space="PSUM") as pwu,
        ):
            # --- persistent tiles -------------------------------------------
            xt = [[pp.tile([128, 2, 1024], BF16, tag=f"x{s}_{k}",
                           name=f"x{s}_{k}")
                   for k in range(2)] for s in range(2)]  # s=0 own, s=1 other
            wpka_sb = pp.tile([128, 2, WPKA_COLS], BF16, tag="wpka")
            wpkb_sb = pp.tile([128, 2, WPKB_COLS], BF16, tag="wpkb")
            rpk2_sb = pp.tile([1, RPK_COLS], BF16, tag="rpk2")
            RC = pp.tile([2, HALF], BF16, tag="RC")      # row0 ekn, row1 ones
            AB = pp.tile([2, C], BF16, tag="AB")         # row0 Bv, row1 A
            tacc = pp.tile([128, 2, 4], F32, tag="tacc")
            uacc = pp.tile([128, 2, 4], F32, tag="uacc")
            tu = pp.tile([128, 2, 2], F32, tag="tu")     # col0 u, col1 t+bqk*u
            tub = pp.tile([128, 2, 2], BF16, tag="tub")
            u2b = pp.tile([128, 2], BF16, tag="u2b")
            t2 = pp.tile([128, 2], F32, tag="t2")
            u2 = pp.tile([128, 2], F32, tag="u2")
            scsel = pp.tile([1, 2], BF16, tag="scsel")   # [0, sc] selector
            wusrc = pp.tile([128, 512], BF16, tag="wusrc")
            atr = pp.tile([1, 1], BF16, tag="atr")       # ACT table-load dummy

            wqcol = lambda q: wpka_sb[:, q, 0:1]
            wkcol = lambda q: wpka_sb[:, q, 1:2]
            wqrep = lambda q: wpka_sb[:, q, 2:130]
            wvt = lambda q: wpkb_sb[:, q, 0:256]
            ident = wpkb_sb[:, 0, 256:WPKB_COLS]
            bvrow = rpk2_sb[0:1, 0:C]
            cgn = rpk2_sb[0:1, C:C + 2]                  # [g*N, 0]

            # --- t=0: DMAs + cheap setup ------------------------------------
            # sync ring: the four x chunks in consumption order (FIFO per
            # ring, so each lands as the previous finishes); scalar ring:
            # weights + small rows (wpka gates the first eqb).
            nc.scalar.dma_start(out=wpka_sb, in_=wpka[:, :, :])
            for s in range(2):
                for k in range(2):
                    nc.sync.dma_start(out=xt[s][k], in_=xd[s][k][:, :, :])
            nc.scalar.dma_start(out=wpkb_sb, in_=wpkb[:, :, :])
            nc.scalar.dma_start(out=rpk2_sb, in_=rpk2[:, :])
            nc.scalar.dma_start(out=RC[1:2, :], in_=rones[:, :])

            nc.vector.memset(wusrc, 0.5)
            nc.vector.memset(scsel, 0.0)
            # ACT function-table load happens at the first activation: trigger
            # it early on a 1-element dummy so it overlaps the DMA wait.
            nc.scalar.activation(out=atr, in_=wusrc[0:1, 0:1], func=ACTF.Copy)

            # PE p-state ramp: dummy matmuls with no DMA dependency.
            def dummy_mm(n, tag):
                for i in range(n):
                    wu = pwu.tile([128, 512], F32, tag="wu", name=f"wu_{tag}_{i}")
                    nc.tensor.matmul(wu, wusrc[:, 0:128], wusrc,
                                     start=True, stop=True)

            dummy_mm(4, "pre")

            # --- phase A: stream x, eq broadcast, t/u reductions, ekn -------
            for sb in range(4):
                s, k = sb // 2, sb % 2
                src = xt[s][k]
                # eq broadcast: [128, 1024] PSUM, 2 blocks x 2 q-chunks
                eqb = pbig.tile([128, 1024], F32, tag="big", name=f"eqb{sb}")
                for half in range(2):
                    blk = slice(512 * half, 512 * (half + 1))
                    for q in range(2):
                        nc.tensor.matmul(eqb[:, blk], wqrep(q), src[:, q, blk],
                                         start=(q == 0), stop=(q == 1))
                dummy_mm(1, f"a{sb}")
                # u accumulation on ACT
                for q in range(2):
                    trsh = tpa.tile([128, 1024], BF16, tag="tr")
                    nc.scalar.activation(out=trsh, in_=src[:, q, :],
                                         func=ACTF.Copy,
                                         accum_out=uacc[:, q, sb:sb + 1])
                # t reduction: fused (eqb+0)*x with free-dim accumulate (DVE)
                for q in range(2):
                    trsh = tpd.tile([128, 1024], BF16, tag="tr")
                    nc.vector.scalar_tensor_tensor(
                        out=trsh, in0=eqb, scalar=0.0, in1=src[:, q, :],
                        op0=OP.add, op1=OP.mult,
                        accum_out=tacc[:, q, sb:sb + 1])

            # --- tail: fold reductions into the AB rows ---------------------
            # ekn rows now: the matmuls keep the PE p-state up through the
            # fold window, and ACT (idle during folds) does the RC copies.
            dummy_mm(2, "t0")
            for k in range(2):
                for half in range(2):
                    blk = slice(512 * half, 512 * (half + 1))
                    gcol = slice(1024 * k + 512 * half,
                                 1024 * k + 512 * half + 512)
                    ekp = psm.tile([1, 512], F32, tag="sm",
                                   name=f"ek{k}_{half}")
                    for q in range(2):
                        nc.tensor.matmul(ekp, wkcol(q), xt[0][k][:, q, blk],
                                         start=(q == 0), stop=(q == 1))
                    nc.scalar.copy(out=RC[0:1, gcol], in_=ekp)
            nc.vector.tensor_reduce(out=u2, in_=uacc, axis=AX.X, op=OP.add)
            nc.vector.tensor_copy(out=u2b, in_=u2)
            nc.vector.tensor_reduce(out=t2, in_=tacc, axis=AX.X, op=OP.add)
            nc.vector.tensor_copy(out=tu[:, :, 0], in_=u2)
            nc.vector.tensor_scalar(out=tu[:, :, 1], in0=u2,
                                    scalar1=bqk, scalar2=None, op0=OP.mult)
            nc.vector.tensor_tensor(out=tu[:, :, 1], in0=tu[:, :, 1],
                                    in1=t2, op=OP.add)
            nc.vector.tensor_copy(out=tub, in_=tu)

            # E = wq_eff . u -> sc = g*E + g*N*bqk into scsel = [0, sc]
            # (runs off u2b so it overlaps the t folds above)
            ep = psm.tile([1, 1], F32, tag="sm", name="ep")
            for q in range(2):
                nc.tensor.matmul(ep, u2b[:, q:q + 1], wqcol(q),
                                 start=(q == 0), stop=(q == 1))
            nc.scalar.activation(out=scsel[0:1, 1:2], in_=ep, func=ACTF.Copy,
                                 scale=g, bias=g * N * bqk)
            # AB rows in one [2, C] PSUM accumulation chain:
            #   row0 (Bv) = g*Wv u        + g*N*bv + 0*bv
            #   row1 (A)  = g*Wv(t+bqk u) + 0      + sc*bv
            P = psm.tile([2, C], F32, tag="sm", name="P")
            for q in range(2):
                nc.tensor.matmul(P, tub[:, q, :], wvt(q),
                                 start=(q == 0), stop=False)
            nc.tensor.matmul(P, cgn, bvrow, start=False, stop=False)
            nc.tensor.matmul(P, scsel, bvrow, start=False, stop=True)
            dummy_mm(2, "t1")
            nc.vector.tensor_copy(out=AB, in_=P)

            # --- phase C: y = x + A + Bv*ekn over own half ------------------
            # blocks (k,q)=(0,0),(1,1): DVE add x+psum; (0,1),(1,0): PE
            # identity-fold + ACT copy.
            dma_eng = [nc.sync, nc.scalar, nc.sync, nc.scalar]
            bi = 0
            for k in range(2):
                for q in range(2):
                    on_dve = (bi % 2 == 0)
                    yps = pbig.tile([128, 1024], F32, tag="big",
                                    name=f"yps{q}_{k}")
                    for half in range(2):
                        blk = slice(512 * half, 512 * (half + 1))
                        gcol = slice(1024 * k + 512 * half,
                                     1024 * k + 512 * half + 512)
                        nc.tensor.matmul(yps[:, blk],
                                         AB[:, 128 * q:128 * (q + 1)],
                                         RC[0:2, gcol], start=True,
                                         stop=on_dve)
                        if not on_dve:
                            nc.tensor.matmul(yps[:, blk], ident,
                                             xt[0][k][:, q, blk],
                                             start=False, stop=True)
                    ysb = yp.tile([128, 1024], BF16, tag="y")
                    if on_dve:
                        nc.vector.tensor_tensor(out=ysb, in0=xt[0][k][:, q, :],
                                                in1=yps, op=OP.add)
                    else:
                        nc.scalar.activation(out=ysb, in_=yps, func=ACTF.Copy)
                    dma_eng[bi].dma_start(
                        out=yout[:, q, 1024 * k:1024 * (k + 1)], in_=ysb)
                    bi += 1
    return nc


def host_prep(x, Wq, bq, Wk, bk, Wc, Wv, bv, gamma):
    """Fold weights on host; build per-core input maps."""
    x = np.asarray(x, dtype=np.float32)
    Wq = np.asarray(Wq, np.float32); bq = np.asarray(bq, np.float32)
    Wk = np.asarray(Wk, np.float32); bk = np.asarray(bk, np.float32)
    Wc = np.asarray(Wc, np.float32)
    Wv = np.asarray(Wv, np.float32); bv = np.asarray(bv, np.float32)
    gamma = float(np.asarray(gamma).reshape(-1)[0])

    wqv, wkv = Wc[:INTER], Wc[INTER:]
    wq_eff = (wqv @ Wq).astype(np.float32)          # [C]
    wk_eff = (wkv @ Wk).astype(np.float32)
    bq_eff = float(wqv @ bq)
    bk_eff = float(wkv @ bk)
    g = gamma / float(N)

    import ml_dtypes
    bf = ml_dtypes.bfloat16

    wpka = np.zeros((128, 2, WPKA_COLS), np.float32)
    wpkb = np.zeros((128, 2, WPKB_COLS), np.float32)
    for q in range(2):
        cs = slice(128 * q, 128 * (q + 1))
        wpka[:, q, 0] = wq_eff[cs]
        wpka[:, q, 1] = wk_eff[cs]
        wpka[:, q, 2:130] = wq_eff[cs][:, None]
        wpkb[:, q, 0:256] = g * Wv.T[cs, :]
    wpkb[:, 0, 256:WPKB_COLS] = np.eye(128, dtype=np.float32)
    wpka = wpka.astype(bf)
    wpkb = wpkb.astype(bf)

    rpk2 = np.concatenate([bv, [g * N, 0.0]]).reshape(1, RPK_COLS).astype(bf)
    rones = np.ones((1, HALF), dtype=bf)

    xr_all = x.reshape(B, C, N)
    xb = xr_all.astype(bf).reshape(B, 2, 128, N)     # [B, q, p, n]
    in_maps = []
    for core in range(NCORES):
        b, half = core // 2, core % 2
        own = slice(HALF * half, HALF * (half + 1))
        other = slice(HALF * (1 - half), HALF * (2 - half))
        im = {
            "wpka": np.ascontiguousarray(wpka),
            "wpkb": np.ascontiguousarray(wpkb),
            "rpk2": np.ascontiguousarray(rpk2),
            "rones": np.ascontiguousarray(rones),
        }
        for s, sl in enumerate([own, other]):
            xs = xb[b][:, :, sl].transpose(1, 0, 2)
            for k in range(2):
                im[f"x{s}{k}"] = np.ascontiguousarray(
                    xs[:, :, 1024 * k:1024 * (k + 1)])
        in_maps.append(im)
    return in_maps, (g, bq_eff, bk_eff)


def assemble(results):
    """Stitch per-core halves into the full output [B, C, H, W]."""
    y = np.empty((B, C, N), dtype=np.float32)
    for core in range(NCORES):
        b, half = core // 2, core % 2
        yo = np.asarray(results[core]["yout"], dtype=np.float32)  # [128,2,2048]
        y[b, :, HALF * half:HALF * (half + 1)] = \
            yo.transpose(1, 0, 2).reshape(C, HALF)
    return y.reshape(B, C, H, W)


def kernel(**inputs):
    install_wait_split()
    in_maps, (g, bq_eff, bk_eff) = host_prep(**inputs)
    nc = build_kernel(g, bq_eff, bk_eff)
    res = run_bass_kernel_spmd(nc, in_maps, core_ids=list(range(NCORES)))
    return assemble(res.results)


# revision 40
# speedup vs baseline: 1.1899x; 1.1899x over previous
"""Trainium2 Bass kernel for Attention_concat (separable PAM attention).

Math (per batch b, N = H*W = 4096):
    eqn[n] = wq_eff . x[:, n]                  (wq_eff = Wq^T Wc[:64])
    ekn[m] = wk_eff . x[:, m]
    y[c, m] = x[c, m] + A[c] + Bv[c] * ekn[m]
with global reductions u = x @ 1, t = x @ eqn and
    Bv = g*Wv u + g*N*bv
    A  = g*Wv (t + (bq_eff+bk_eff) u) + bv*(g*E + g*N*(bq_eff+bk_eff))
    E  = wq_eff . u,   g = gamma / N

Precision: the attention correction is ~1.5e-4 of |y|, so the whole pipeline
runs in bf16 (x is loaded bf16, y stored bf16); measured rel-to-scale error
~3e-3 vs the 2e-2 gate.

Sharding: 2 cores per batch, each handles half the spatial columns. Each core
redundantly computes the global reductions over the full x[b] (own half + a
bf16 copy of the other half), then writes its own 2048 output columns.

Engine/DMA split: inputs ride both HWDGE rings (sync: x own half; scalar: the
weight pack first, then x other half) so the eqb-gating weights land early.
PE broadcasts eqn into PSUM (stationary wq_eff replicated along the free dim)
and computes ekn rows; DVE does the t-reduction via scalar_tensor_tensor with
accum_out; ACT accumulates u via activation-Copy accum_out; ekn PSUM->RC
copies split ACT/DVE. The A/Bv rows are assembled in one [2,C] PSUM
accumulation chain (tub stationary + [gN,0]/[0,sc] selector rows against the
bv row) — no cross-partition moves. Phase C: rank-2 AB x RC matmul per
512-block; two blocks finish as DVE adds (x + psum), two as PE identity-fold
plus ACT copy, then 4 output DMAs alternating rings. Dummy matmuls keep the
PE p-state up across idle windows.

Module-level workarounds (this container's walrus accepts only one sync-wait
per instruction): extra waits are hoisted onto single-wait NoOps at BIR level,
and the Tile tail drain is rebuilt the same way.
"""

import json as _json

import numpy as np

import concourse.bass as bass
import concourse.bass2jax as _b2j
import concourse.bass_utils as _bu
import concourse.mybir as mybir
import concourse.tile as tile
from concourse.bass_utils import run_bass_kernel_spmd
from concourse.vector_clock import ScopedClock, VectorClock

B, C, H, W = 4, 256, 64, 64
N = H * W            # 4096
INTER = C // 4       # 64
NCORES = 8
HALF = N // 2        # 2048 output columns per core
F32 = mybir.dt.float32
BF16 = mybir.dt.bfloat16
AX = mybir.AxisListType
OP = mybir.AluOpType
ACTF = mybir.ActivationFunctionType

# wpka free-dim layout (per q chunk): [0]=wq_eff col, [1]=wk_eff col,
# [2:130]=wq_eff replicated 128  (gates phase A -> lands first)
WPKA_COLS = 130
# wpkb: [0:256]=g*Wv^T, [256:384]=identity in q=0  (tail-only -> lands last)
WPKB_COLS = 384
# rpk2 row-pack: [0:256]=bv, [256]=g*N, [257]=0
RPK_COLS = 258


def _split_multi_waits(bir: dict) -> dict:
    """The nix walrus accepts only ONE sync-wait command per instruction.
    Hoist extra waits onto preceding single-wait NoOps on the same engine
    (sequencers execute in program order, so semantics are unchanged)."""
    ctr = 0
    for fn in bir.get("functions", []):
        for blk in fn.get("blocks", []):
            insts = blk.get("instructions")
            if not insts:
                continue
            out = []
            for inst in insts:
                si = inst.get("sync_info") or {}
                waits = si.get("on_wait") or []
                if len(waits) > 1 and inst.get("engine", "Unassigned") != "Unassigned":
                    for w in waits[:-1]:
                        ctr += 1
                        out.append({
                            "debug": inst.get("debug", 0),
                            "engine": inst["engine"],
                            "ins": [], "outs": [],
                            "name": f"{inst['name']}-ws{ctr}",
                            "opcode": "NoOp",
                            "sync_info": {"on_update": [], "on_wait": [w]},
                        })
                    si["on_wait"] = [waits[-1]]
                out.append(inst)
            blk["instructions"] = out
    return bir


_WAIT_SPLIT_DONE = False


def install_wait_split():
    global _WAIT_SPLIT_DONE
    if _WAIT_SPLIT_DONE:
        return
    orig = _bu.compile_bir_kernel

    def wrapped(bir_json, *a, **kw):
        d = _json.loads(bir_json)
        _split_multi_waits(d)
        return orig(_json.dumps(d).encode(), *a, **kw)

    _bu.compile_bir_kernel = wrapped
    _b2j.compile_bir_kernel = wrapped
    _WAIT_SPLIT_DONE = True


class SplitDrainTileContext(tile.TileContext):
    """Tail fix for the same 1-wait walrus limit: park the global-clock waits
    on single-wait Nops spread across all five engines (they wait in
    parallel), then a wait-free drain + the usual barrier/reset."""

    def _drain_and_barrier(self, tick_clock, wait_clock):
        gc = tick_clock.global_clock
        nprocs = len(gc)
        engines = [self.nc.sync, self.nc.vector, self.nc.scalar,
                   self.nc.gpsimd, self.nc.tensor]
        idx = 0
        for proc in range(nprocs):
            if gc[proc] > 0:
                eng = engines[idx % len(engines)]
                idx += 1
                nop = eng.nop(nofuse=True, hint=f"tail_wait_p{proc}")
                vc = VectorClock([0] * nprocs)
                vc.require_at_least(proc, gc[proc])
                wait_clock.add_sem_waits(nop.ins, ScopedClock({None: vc}))
        self.nc.sync.drain()
        self.nc.all_engine_barrier()
        assert self.sems is not None
        popped = self.nc._tile_sem_poison_stack.pop()
        assert popped is self._sem_poison
        self.nc.clear_and_free_semaphores(list(self.sems.allocated().values()))
        self.nc.all_engine_barrier()


def build_kernel(g: float, bq_eff: float, bk_eff: float):
    """Build the per-core Bass program. g = gamma/N."""
    bqk = bq_eff + bk_eff
    nc = bass.Bass()
    xd = [[nc.dram_tensor(f"x{s}{k}", [128, 2, 1024], BF16,
                          kind="ExternalInput")
           for k in range(2)] for s in range(2)]
    wpka = nc.dram_tensor("wpka", [128, 2, WPKA_COLS], BF16, kind="ExternalInput")
    wpkb = nc.dram_tensor("wpkb", [128, 2, WPKB_COLS], BF16, kind="ExternalInput")
    rpk2 = nc.dram_tensor("rpk2", [1, RPK_COLS], BF16, kind="ExternalInput")
    rones = nc.dram_tensor("rones", [1, HALF], BF16, kind="ExternalInput")
    yout = nc.dram_tensor("yout", [128, 2, HALF], BF16, kind="ExternalOutput")

    with SplitDrainTileContext(nc) as tc:
        with (
            tc.tile_pool(name="persist", bufs=1) as pp,
            tc.tile_pool(name="trasha", bufs=1) as tpa,
            tc.tile_pool(name="trashd", bufs=1) as tpd,
            tc.tile_pool(name="ypool", bufs=4) as yp,
            tc.tile_pool(name="psm", bufs=2, space="PSUM") as psm,
            tc.tile_pool(name="pbig", bufs=2, space="PSUM") as pbig,
            tc.tile_pool(name="pwu", bufs=1, space="PSUM") as pwu,
        ):
            # --- persistent tiles -------------------------------------------
            xts = [pp.tile([128, 2, HALF], BF16, tag=f"x{s}", name=f"x{s}")
                   for s in range(2)]                    # s=0 own, s=1 other
            xt = [[xts[s][:, :, 1024 * k:1024 * (k + 1)] for k in range(2)]
                  for s in range(2)]
            wpka_sb = pp.tile([128, 2, WPKA_COLS], BF16, tag="wpka")
            wpkb_sb = pp.tile([128, 2, WPKB_COLS], BF16, tag="wpkb")
            rpk2_sb = pp.tile([1, RPK_COLS], BF16, tag="rpk2")
            RC = pp.tile([2, HALF], BF16, tag="RC")      # row0 ekn, row1 ones
            AB = pp.tile([2, C], BF16, tag="AB")         # row0 Bv, row1 A
            tacc = pp.tile([128, 2, 4], F32, tag="tacc")
            uacc = pp.tile([128, 2, 2], F32, tag="uacc")
            tu = pp.tile([128, 2, 2], F32, tag="tu")     # col0 u, col1 t+bqk*u
            tub = pp.tile([128, 2, 2], BF16, tag="tub")
            u2b = pp.tile([128, 2], BF16, tag="u2b")
            t2 = pp.tile([128, 2], F32, tag="t2")
            u2 = pp.tile([128, 2], F32, tag="u2")
            scsel = pp.tile([1, 2], BF16, tag="scsel")   # [0, sc] selector
            wusrc = pp.tile([128, 512], BF16, tag="wusrc")
            atr = pp.tile([1, 1], BF16, tag="atr")       # ACT table-load dummy

            wqcol = lambda q: wpka_sb[:, q, 0:1]
            wkcol = lambda q: wpka_sb[:, q, 1:2]
            wqrep = lambda q: wpka_sb[:, q, 2:130]
            wvt = lambda q: wpkb_sb[:, q, 0:256]
            ident = wpkb_sb[:, 0, 256:WPKB_COLS]
            bvrow = rpk2_sb[0:1, 0:C]
            cgn = rpk2_sb[0:1, C:C + 2]                  # [g*N, 0]

            # --- t=0: DMAs + cheap setup ------------------------------------
            # sync ring: the four x chunks in consumption order (FIFO per
            # ring, so each lands as the previous finishes); scalar ring:
            # weights + small rows (wpka gates the first eqb).
            nc.scalar.dma_start(out=wpka_sb, in_=wpka[:, :, :])
            for s in range(2):
                for k in range(2):
                    nc.sync.dma_start(out=xts[s][:, :, 1024 * k:1024 * (k + 1)],
                                      in_=xd[s][k][:, :, :])
            nc.scalar.dma_start(out=wpkb_sb, in_=wpkb[:, :, :])
            nc.scalar.dma_start(out=rpk2_sb, in_=rpk2[:, :])
            nc.scalar.dma_start(out=RC[1:2, :], in_=rones[:, :])

            nc.vector.memset(wusrc, 0.5)
            nc.vector.memset(scsel, 0.0)
            # ACT function-table load happens at the first activation: trigger
            # it early on a 1-element dummy so it overlaps the DMA wait.
            nc.scalar.activation(out=atr, in_=wusrc[0:1, 0:1], func=ACTF.Copy)

            # PE p-state ramp: dummy matmuls with no DMA dependency.
            def dummy_mm(n, tag):
                for i in range(n):
                    wu = pwu.tile([128, 512], F32, tag="wu", name=f"wu_{tag}_{i}")
                    nc.tensor.matmul(wu, wusrc[:, 0:128], wusrc,
                                     start=True, stop=True)

            dummy_mm(4, "pre")

            # --- phase A: stream x, eq broadcast, t/u reductions, ekn -------
            for sb in range(4):
                s, k = sb // 2, sb % 2
                src = xt[s][k]
                # eq broadcast: [128, 1024] PSUM, 2 blocks x 2 q-chunks
                eqb = pbig.tile([128, 1024], F32, tag="big", name=f"eqb{sb}")
                for half in range(2):
                    blk = slice(512 * half, 512 * (half + 1))
                    for q in range(2):
                        nc.tensor.matmul(eqb[:, blk], wqrep(q), src[:, q, blk],
                                         start=(q == 0), stop=(q == 1))
                dummy_mm(1, f"a{sb}")
                # u accumulation on ACT: one [128, 2048] pass per (s, q)
                if k == 1:
                    for q in range(2):
                        trsh = tpa.tile([128, 2048], BF16, tag="tr")
                        nc.scalar.activation(out=trsh, in_=xts[s][:, q, :],
                                             func=ACTF.Copy,
                                             accum_out=uacc[:, q, s:s + 1])
                # t reduction: fused (eqb+0)*x with free-dim accumulate (DVE)
                for q in range(2):
                    trsh = tpd.tile([128, 1024], BF16, tag="tr")
                    nc.vector.scalar_tensor_tensor(
                        out=trsh, in0=eqb, scalar=0.0, in1=src[:, q, :],
                        op0=OP.add, op1=OP.mult,
                        accum_out=tacc[:, q, sb:sb + 1])

            # --- tail: fold reductions into the AB rows ---------------------
            # ekn rows now: the matmuls keep the PE p-state up through the
            # fold window, and ACT (idle during folds) does the RC copies.
            dummy_mm(14, "t0")
            for k in range(2):
                for half in range(2):
                    blk = slice(512 * half, 512 * (half + 1))
                    gcol = slice(1024 * k + 512 * half,
                                 1024 * k + 512 * half + 512)
                    ekp = psm.tile([1, 512], F32, tag="sm",
                                   name=f"ek{k}_{half}")
                    for q in range(2):
                        nc.tensor.matmul(ekp, wkcol(q), xt[0][k][:, q, blk],
                                         start=(q == 0), stop=(q == 1))
                    if half == 0:
                        nc.scalar.copy(out=RC[0:1, gcol], in_=ekp)
                    else:
                        nc.vector.tensor_copy(out=RC[0:1, gcol], in_=ekp)
            nc.vector.tensor_reduce(out=u2, in_=uacc, axis=AX.X, op=OP.add)
            nc.vector.tensor_copy(out=u2b, in_=u2)
            nc.vector.tensor_reduce(out=t2, in_=tacc, axis=AX.X, op=OP.add)
            nc.vector.tensor_copy(out=tu[:, :, 0], in_=u2)
            nc.vector.tensor_scalar(out=tu[:, :, 1], in0=u2,
                                    scalar1=bqk, scalar2=None, op0=OP.mult)
            nc.vector.tensor_tensor(out=tu[:, :, 1], in0=tu[:, :, 1],
                                    in1=t2, op=OP.add)
            nc.vector.tensor_copy(out=tub, in_=tu)

            # E = wq_eff . u -> sc = g*E + g*N*bqk into scsel = [0, sc]
            # (runs off u2b so it overlaps the t folds above)
            ep = psm.tile([1, 1], F32, tag="sm", name="ep")
            for q in range(2):
                nc.tensor.matmul(ep, u2b[:, q:q + 1], wqcol(q),
                                 start=(q == 0), stop=(q == 1))
            nc.scalar.activation(out=scsel[0:1, 1:2], in_=ep, func=ACTF.Copy,
                                 scale=g, bias=g * N * bqk)
            # AB rows in one [2, C] PSUM accumulation chain:
            #   row0 (Bv) = g*Wv u        + g*N*bv + 0*bv
            #   row1 (A)  = g*Wv(t+bqk u) + 0      + sc*bv
            P = psm.tile([2, C], F32, tag="sm", name="P")
            for q in range(2):
                nc.tensor.matmul(P, tub[:, q, :], wvt(q),
                                 start=(q == 0), stop=False)
            nc.tensor.matmul(P, cgn, bvrow, start=False, stop=False)
            nc.tensor.matmul(P, scsel, bvrow, start=False, stop=True)
            dummy_mm(2, "t1")
            nc.vector.tensor_copy(out=AB, in_=P)

            # --- phase C: y = x + A + Bv*ekn over own half ------------------
            # blocks (k,q)=(0,0),(1,1): DVE add x+psum; (0,1),(1,0): PE
            # identity-fold + ACT copy.
            dma_eng = [nc.sync, nc.scalar, nc.sync, nc.scalar]
            bi = 0
            for k in range(2):
                for q in range(2):
                    on_dve = (bi % 2 == 0)
                    yps = pbig.tile([128, 1024], F32, tag="big",
                                    name=f"yps{q}_{k}")
                    for half in range(2):
                        blk = slice(512 * half, 512 * (half + 1))
                        gcol = slice(1024 * k + 512 * half,
                                     1024 * k + 512 * half + 512)
                        nc.tensor.matmul(yps[:, blk],
                                         AB[:, 128 * q:128 * (q + 1)],
                                         RC[0:2, gcol], start=True,
                                         stop=on_dve)
                        if not on_dve:
                            nc.tensor.matmul(yps[:, blk], ident,
                                             xt[0][k][:, q, blk],
                                             start=False, stop=True)
                    ysb = yp.tile([128, 1024], BF16, tag="y")
                    if on_dve:
                        nc.vector.tensor_tensor(out=ysb, in0=xt[0][k][:, q, :],
                                                in1=yps, op=OP.add)
                    else:
                        nc.scalar.activation(out=ysb, in_=yps, func=ACTF.Copy)
                    dma_eng[bi].dma_start(
                        out=yout[:, q, 1024 * k:1024 * (k + 1)], in_=ysb)
                    bi += 1
    return nc


def host_prep(x, Wq, bq, Wk, bk, Wc, Wv, bv, gamma):
    """Fold weights on host; build per-core input maps."""
    x = np.asarray(x, dtype=np.float32)
    Wq = np.asarray(Wq, np.float32); bq = np.asarray(bq, np.float32)
    Wk = np.asarray(Wk, np.float32); bk = np.asarray(bk, np.float32)
    Wc = np.asarray(Wc, np.float32)
    Wv = np.asarray(Wv, np.float32); bv = np.asarray(bv, np.float32)
    gamma = float(np.asarray(gamma).reshape(-1)[0])

    wqv, wkv = Wc[:INTER], Wc[INTER:]
    wq_eff = (wqv @ Wq).astype(np.float32)          # [C]
    wk_eff = (wkv @ Wk).astype(np.float32)
    bq_eff = float(wqv @ bq)
    bk_eff = float(wkv @ bk)
    g = gamma / float(N)

    import ml_dtypes
    bf = ml_dtypes.bfloat16

    wpka = np.zeros((128, 2, WPKA_COLS), np.float32)
    wpkb = np.zeros((128, 2, WPKB_COLS), np.float32)
    for q in range(2):
        cs = slice(128 * q, 128 * (q + 1))
        wpka[:, q, 0] = wq_eff[cs]
        wpka[:, q, 1] = wk_eff[cs]
        wpka[:, q, 2:130] = wq_eff[cs][:, None]
        wpkb[:, q, 0:256] = g * Wv.T[cs, :]
    wpkb[:, 0, 256:WPKB_COLS] = np.eye(128, dtype=np.float32)
    wpka = wpka.astype(bf)
    wpkb = wpkb.astype(bf)

    rpk2 = np.concatenate([bv, [g * N, 0.0]]).reshape(1, RPK_COLS).astype(bf)
    rones = np.ones((1, HALF), dtype=bf)

    xr_all = x.reshape(B, C, N)
    xb = xr_all.astype(bf).reshape(B, 2, 128, N)     # [B, q, p, n]
    in_maps = []
    for core in range(NCORES):
        b, half = core // 2, core % 2
        own = slice(HALF * half, HALF * (half + 1))
        other = slice(HALF * (1 - half), HALF * (2 - half))
        im = {
            "wpka": np.ascontiguousarray(wpka),
            "wpkb": np.ascontiguousarray(wpkb),
            "rpk2": np.ascontiguousarray(rpk2),
            "rones": np.ascontiguousarray(rones),
        }
        for s, sl in enumerate([own, other]):
            xs = xb[b][:, :, sl].transpose(1, 0, 2)
            for k in range(2):
                im[f"x{s}{k}"] = np.ascontiguousarray(
                    xs[:, :, 1024 * k:1024 * (k + 1)])
        in_maps.append(im)
    return in_maps, (g, bq_eff, bk_eff)


def assemble(results):
    """Stitch per-core halves into the full output [B, C, H, W]."""
    y = np.empty((B, C, N), dtype=np.float32)
    for core in range(NCORES):
        b, half = core // 2, core % 2
        yo = np.asarray(results[core]["yout"], dtype=np.float32)  # [128,2,2048]
        y[b, :, HALF * half:HALF * (half + 1)] = \
            yo.transpose(1, 0, 2).reshape(C, HALF)
    return y.reshape(B, C, H, W)


def kernel(**inputs):
    install_wait_split()
    in_maps, (g, bq_eff, bk_eff) = host_prep(**inputs)
    nc = build_kernel(g, bq_eff, bk_eff)
    res = run_bass_kernel_spmd(nc, in_maps, core_ids=list(range(NCORES)))
    return assemble(res.results)


# revision 41
# speedup vs baseline: 1.2155x; 1.0215x over previous
"""Trainium2 Bass kernel for Attention_concat (separable PAM attention).

Math (per batch b, N = H*W = 4096):
    eqn[n] = wq_eff . x[:, n]                  (wq_eff = Wq^T Wc[:64])
    ekn[m] = wk_eff . x[:, m]
    y[c, m] = x[c, m] + A[c] + Bv[c] * ekn[m]
with global reductions u = x @ 1, t = x @ eqn and
    Bv = g*Wv u + g*N*bv
    A  = g*Wv (t + (bq_eff+bk_eff) u) + bv*(g*E + g*N*(bq_eff+bk_eff))
    E  = wq_eff . u,   g = gamma / N

Precision: the attention correction is ~1.5e-4 of |y|, so the whole pipeline
runs in bf16 (x is loaded bf16, y stored bf16); measured rel-to-scale error
~3e-3 vs the 2e-2 gate.

Sharding: 2 cores per batch, each handles half the spatial columns. Each core
redundantly computes the global reductions over the full x[b] (own half + a
bf16 copy of the other half), then writes its own 2048 output columns.

Engine/DMA split: inputs ride both HWDGE rings (sync: x own half; scalar: the
weight pack first, then x other half) so the eqb-gating weights land early.
PE broadcasts eqn into PSUM (stationary wq_eff replicated along the free dim)
and computes ekn rows; DVE does the t-reduction via scalar_tensor_tensor with
accum_out; ACT accumulates u via activation-Copy accum_out; ekn PSUM->RC
copies split ACT/DVE. The A/Bv rows are assembled in one [2,C] PSUM
accumulation chain (tub stationary + [gN,0]/[0,sc] selector rows against the
bv row) — no cross-partition moves. Phase C: rank-2 AB x RC matmul per
512-block; two blocks finish as DVE adds (x + psum), two as PE identity-fold
plus ACT copy, then 4 output DMAs alternating rings. Dummy matmuls keep the
PE p-state up across idle windows.

Module-level workarounds (this container's walrus accepts only one sync-wait
per instruction): extra waits are hoisted onto single-wait NoOps at BIR level,
and the Tile tail drain is rebuilt the same way.
"""

import json as _json

import numpy as np

import concourse.bass as bass
import concourse.bass2jax as _b2j
import concourse.bass_utils as _bu
import concourse.mybir as mybir
import concourse.tile as tile
from concourse.bass_utils import run_bass_kernel_spmd
from concourse.vector_clock import ScopedClock, VectorClock

B, C, H, W = 4, 256, 64, 64
N = H * W            # 4096
INTER = C // 4       # 64
NCORES = 8
HALF = N // 2        # 2048 output columns per core
F32 = mybir.dt.float32
BF16 = mybir.dt.bfloat16
AX = mybir.AxisListType
OP = mybir.AluOpType
ACTF = mybir.ActivationFunctionType

# wpka free-dim layout (per q chunk): [0]=wq_eff col, [1]=wk_eff col,
# [2:130]=wq_eff replicated 128  (gates phase A -> lands first)
WPKA_COLS = 130
# wpkb: [0:256]=g*Wv^T, [256:384]=identity in q=0  (tail-only -> lands last)
WPKB_COLS = 384
# rpk2 row-pack: [0:256]=bv, [256]=g*N, [257]=0
RPK_COLS = 258


def _split_multi_waits(bir: dict) -> dict:
    """The nix walrus accepts only ONE sync-wait command per instruction.
    Hoist extra waits onto preceding single-wait NoOps on the same engine
    (sequencers execute in program order, so semantics are unchanged)."""
    ctr = 0
    for fn in bir.get("functions", []):
        for blk in fn.get("blocks", []):
            insts = blk.get("instructions")
            if not insts:
                continue
            out = []
            for inst in insts:
                si = inst.get("sync_info") or {}
                waits = si.get("on_wait") or []
                if len(waits) > 1 and inst.get("engine", "Unassigned") != "Unassigned":
                    for w in waits[:-1]:
                        ctr += 1
                        out.append({
                            "debug": inst.get("debug", 0),
                            "engine": inst["engine"],
                            "ins": [], "outs": [],
                            "name": f"{inst['name']}-ws{ctr}",
                            "opcode": "NoOp",
                            "sync_info": {"on_update": [], "on_wait": [w]},
                        })
                    si["on_wait"] = [waits[-1]]
                out.append(inst)
            blk["instructions"] = out
    return bir


_WAIT_SPLIT_DONE = False


def install_wait_split():
    global _WAIT_SPLIT_DONE
    if _WAIT_SPLIT_DONE:
        return
    orig = _bu.compile_bir_kernel

    def wrapped(bir_json, *a, **kw):
        d = _json.loads(bir_json)
        _split_multi_waits(d)
        return orig(_json.dumps(d).encode(), *a, **kw)

    _bu.compile_bir_kernel = wrapped
    _b2j.compile_bir_kernel = wrapped
    _WAIT_SPLIT_DONE = True


class SplitDrainTileContext(tile.TileContext):
    """Tail fix for the same 1-wait walrus limit: park the global-clock waits
    on single-wait Nops spread across all five engines (they wait in
    parallel), then a wait-free drain + the usual barrier/reset."""

    def _drain_and_barrier(self, tick_clock, wait_clock):
        gc = tick_clock.global_clock
        nprocs = len(gc)
        engines = [self.nc.sync, self.nc.vector, self.nc.scalar,
                   self.nc.gpsimd, self.nc.tensor]
        idx = 0
        for proc in range(nprocs):
            if gc[proc] > 0:
                eng = engines[idx % len(engines)]
                idx += 1
                nop = eng.nop(nofuse=True, hint=f"tail_wait_p{proc}")
                vc = VectorClock([0] * nprocs)
                vc.require_at_least(proc, gc[proc])
                wait_clock.add_sem_waits(nop.ins, ScopedClock({None: vc}))
        self.nc.sync.drain()
        self.nc.all_engine_barrier()
        assert self.sems is not None
        popped = self.nc._tile_sem_poison_stack.pop()
        assert popped is self._sem_poison
        self.nc.clear_and_free_semaphores(list(self.sems.allocated().values()))
        self.nc.all_engine_barrier()


def build_kernel(g: float, bq_eff: float, bk_eff: float):
    """Build the per-core Bass program. g = gamma/N."""
    bqk = bq_eff + bk_eff
    nc = bass.Bass()
    xd = [[nc.dram_tensor(f"x{s}{k}", [128, 2, 1024], BF16,
                          kind="ExternalInput")
           for k in range(2)] for s in range(2)]
    wpka = nc.dram_tensor("wpka", [128, 2, WPKA_COLS], BF16, kind="ExternalInput")
    wpkb = nc.dram_tensor("wpkb", [128, 2, WPKB_COLS], BF16, kind="ExternalInput")
    rpk2 = nc.dram_tensor("rpk2", [1, RPK_COLS], BF16, kind="ExternalInput")
    rones = nc.dram_tensor("rones", [1, HALF], BF16, kind="ExternalInput")
    yout = nc.dram_tensor("yout", [128, 2, HALF], BF16, kind="ExternalOutput")

    with SplitDrainTileContext(nc) as tc:
        with (
            tc.tile_pool(name="persist", bufs=1) as pp,
            tc.tile_pool(name="trasha", bufs=1) as tpa,
            tc.tile_pool(name="trashd", bufs=1) as tpd,
            tc.tile_pool(name="ypool", bufs=4) as yp,
            tc.tile_pool(name="psm", bufs=2, space="PSUM") as psm,
            tc.tile_pool(name="pbig", bufs=2, space="PSUM") as pbig,
            tc.tile_pool(name="pwu", bufs=1, space="PSUM") as pwu,
        ):
            # --- persistent tiles -------------------------------------------
            xts = [pp.tile([128, 2, HALF], BF16, tag=f"x{s}", name=f"x{s}")
                   for s in range(2)]                    # s=0 own, s=1 other
            xt = [[xts[s][:, :, 1024 * k:1024 * (k + 1)] for k in range(2)]
                  for s in range(2)]
            wpka_sb = pp.tile([128, 2, WPKA_COLS], BF16, tag="wpka")
            wpkb_sb = pp.tile([128, 2, WPKB_COLS], BF16, tag="wpkb")
            rpk2_sb = pp.tile([1, RPK_COLS], BF16, tag="rpk2")
            RC = pp.tile([2, HALF], BF16, tag="RC")      # row0 ekn, row1 ones
            AB = pp.tile([2, C], BF16, tag="AB")         # row0 Bv, row1 A
            tacc = pp.tile([128, 2, 4], F32, tag="tacc")
            uacc = pp.tile([128, 2, 2], F32, tag="uacc")
            tu = pp.tile([128, 2, 2], F32, tag="tu")     # col0 u, col1 t+bqk*u
            tub = pp.tile([128, 2, 2], BF16, tag="tub")
            u2b = pp.tile([128, 2], BF16, tag="u2b")
            t2 = pp.tile([128, 2], F32, tag="t2")
            u2 = pp.tile([128, 2], F32, tag="u2")
            scsel = pp.tile([1, 2], BF16, tag="scsel")   # [0, sc] selector
            wusrc = pp.tile([128, 512], BF16, tag="wusrc")
            atr = pp.tile([1, 1], BF16, tag="atr")       # ACT table-load dummy

            wqcol = lambda q: wpka_sb[:, q, 0:1]
            wkcol = lambda q: wpka_sb[:, q, 1:2]
            wqrep = lambda q: wpka_sb[:, q, 2:130]
            wvt = lambda q: wpkb_sb[:, q, 0:256]
            ident = wpkb_sb[:, 0, 256:WPKB_COLS]
            bvrow = rpk2_sb[0:1, 0:C]
            cgn = rpk2_sb[0:1, C:C + 2]                  # [g*N, 0]

            # --- t=0: DMAs + cheap setup ------------------------------------
            # sync ring: the four x chunks in consumption order (FIFO per
            # ring, so each lands as the previous finishes); scalar ring:
            # weights + small rows (wpka gates the first eqb).
            nc.scalar.dma_start(out=wpka_sb, in_=wpka[:, :, :])
            for s in range(2):
                for k in range(2):
                    nc.sync.dma_start(out=xts[s][:, :, 1024 * k:1024 * (k + 1)],
                                      in_=xd[s][k][:, :, :])
            nc.scalar.dma_start(out=wpkb_sb, in_=wpkb[:, :, :])
            nc.scalar.dma_start(out=rpk2_sb, in_=rpk2[:, :])
            nc.scalar.dma_start(out=RC[1:2, :], in_=rones[:, :])

            nc.vector.memset(wusrc, 0.5)
            nc.vector.memset(scsel, 0.0)
            # ACT function-table load happens at the first activation: trigger
            # it early on a 1-element dummy so it overlaps the DMA wait.
            nc.scalar.activation(out=atr, in_=wusrc[0:1, 0:1], func=ACTF.Copy)

            # PE p-state ramp: dummy matmuls with no DMA dependency.
            def dummy_mm(n, tag):
                for i in range(n):
                    wu = pwu.tile([128, 512], F32, tag="wu", name=f"wu_{tag}_{i}")
                    nc.tensor.matmul(wu, wusrc[:, 0:128], wusrc,
                                     start=True, stop=True)

            dummy_mm(4, "pre")

            # --- phase A: stream x, eq broadcast, t/u reductions, ekn -------
            for sb in range(4):
                s, k = sb // 2, sb % 2
                src = xt[s][k]
                # eq broadcast: [128, 1024] PSUM, 2 blocks x 2 q-chunks
                eqb = pbig.tile([128, 1024], F32, tag="big", name=f"eqb{sb}")
                for half in range(2):
                    blk = slice(512 * half, 512 * (half + 1))
                    for q in range(2):
                        nc.tensor.matmul(eqb[:, blk], wqrep(q), src[:, q, blk],
                                         start=(q == 0), stop=(q == 1))
                dummy_mm(1, f"a{sb}")
                # u accumulation on ACT: one [128, 2048] pass per (s, q)
                if k == 1:
                    for q in range(2):
                        trsh = tpa.tile([128, 2048], BF16, tag="tr")
                        nc.scalar.activation(out=trsh, in_=xts[s][:, q, :],
                                             func=ACTF.Copy,
                                             accum_out=uacc[:, q, s:s + 1])
                # t reduction: fused (eqb+0)*x with free-dim accumulate (DVE)
                for q in range(2):
                    trsh = tpd.tile([128, 1024], BF16, tag="tr")
                    nc.vector.scalar_tensor_tensor(
                        out=trsh, in0=eqb, scalar=0.0, in1=src[:, q, :],
                        op0=OP.add, op1=OP.mult,
                        accum_out=tacc[:, q, sb:sb + 1])

            # --- tail: fold reductions into the AB rows ---------------------
            # ekn rows now: the matmuls keep the PE p-state up through the
            # fold window, and ACT (idle during folds) does the RC copies.
            dummy_mm(14, "t0")
            for k in range(2):
                for half in range(2):
                    blk = slice(512 * half, 512 * (half + 1))
                    gcol = slice(1024 * k + 512 * half,
                                 1024 * k + 512 * half + 512)
                    ekp = psm.tile([1, 512], F32, tag="sm",
                                   name=f"ek{k}_{half}")
                    for q in range(2):
                        nc.tensor.matmul(ekp, wkcol(q), xt[0][k][:, q, blk],
                                         start=(q == 0), stop=(q == 1))
                    nc.scalar.copy(out=RC[0:1, gcol], in_=ekp)
            nc.vector.tensor_reduce(out=u2, in_=uacc, axis=AX.X, op=OP.add)
            nc.vector.tensor_copy(out=u2b, in_=u2)
            nc.vector.tensor_reduce(out=t2, in_=tacc, axis=AX.X, op=OP.add)
            nc.vector.tensor_copy(out=tu[:, :, 0], in_=u2)
            nc.vector.tensor_scalar(out=tu[:, :, 1], in0=u2,
                                    scalar1=bqk, scalar2=None, op0=OP.mult)
            nc.vector.tensor_tensor(out=tu[:, :, 1], in0=tu[:, :, 1],
                                    in1=t2, op=OP.add)
            nc.vector.tensor_copy(out=tub, in_=tu)

            # E = wq_eff . u -> sc = g*E + g*N*bqk into scsel = [0, sc]
            # (runs off u2b so it overlaps the t folds above)
            ep = psm.tile([1, 1], F32, tag="sm", name="ep")
            for q in range(2):
                nc.tensor.matmul(ep, u2b[:, q:q + 1], wqcol(q),
                                 start=(q == 0), stop=(q == 1))
            nc.scalar.activation(out=scsel[0:1, 1:2], in_=ep, func=ACTF.Copy,
                                 scale=g, bias=g * N * bqk)
            # AB rows in one [2, C] PSUM accumulation chain:
            #   row0 (Bv) = g*Wv u        + g*N*bv + 0*bv
            #   row1 (A)  = g*Wv(t+bqk u) + 0      + sc*bv
            P = psm.tile([2, C], F32, tag="sm", name="P")
            for q in range(2):
                nc.tensor.matmul(P, tub[:, q, :], wvt(q),
                                 start=(q == 0), stop=False)
            nc.tensor.matmul(P, cgn, bvrow, start=False, stop=False)
            nc.tensor.matmul(P, scsel, bvrow, start=False, stop=True)
            dummy_mm(2, "t1")
            nc.vector.tensor_copy(out=AB, in_=P)

            # --- phase C: y = x + A + Bv*ekn over own half ------------------
            # blocks (k,q)=(0,0),(1,1): DVE add x+psum; (0,1),(1,0): PE
            # identity-fold + ACT copy.
            dma_eng = [nc.sync, nc.scalar, nc.sync, nc.scalar]
            bi = 0
            for k in range(2):
                for q in range(2):
                    on_dve = (bi % 2 == 0)
                    yps = pbig.tile([128, 1024], F32, tag="big",
                                    name=f"yps{q}_{k}")
                    for half in range(2):
                        blk = slice(512 * half, 512 * (half + 1))
                        gcol = slice(1024 * k + 512 * half,
                                     1024 * k + 512 * half + 512)
                        nc.tensor.matmul(yps[:, blk],
                                         AB[:, 128 * q:128 * (q + 1)],
                                         RC[0:2, gcol], start=True,
                                         stop=on_dve)
                        if not on_dve:
                            nc.tensor.matmul(yps[:, blk], ident,
                                             xt[0][k][:, q, blk],
                                             start=False, stop=True)
                    ysb = yp.tile([128, 1024], BF16, tag="y")
                    if on_dve:
                        nc.vector.tensor_tensor(out=ysb, in0=xt[0][k][:, q, :],
                                                in1=yps, op=OP.add)
                    else:
                        nc.scalar.activation(out=ysb, in_=yps, func=ACTF.Copy)
                    dma_eng[bi].dma_start(
                        out=yout[:, q, 1024 * k:1024 * (k + 1)], in_=ysb)
                    bi += 1
    return nc


def host_prep(x, Wq, bq, Wk, bk, Wc, Wv, bv, gamma):
    """Fold weights on host; build per-core input maps."""
    x = np.asarray(x, dtype=np.float32)
    Wq = np.asarray(Wq, np.float32); bq = np.asarray(bq, np.float32)
    Wk = np.asarray(Wk, np.float32); bk = np.asarray(bk, np.float32)
    Wc = np.asarray(Wc, np.float32)
    Wv = np.asarray(Wv, np.float32); bv = np.asarray(bv, np.float32)
    gamma = float(np.asarray(gamma).reshape(-1)[0])

    wqv, wkv = Wc[:INTER], Wc[INTER:]
    wq_eff = (wqv @ Wq).astype(np.float32)          # [C]
    wk_eff = (wkv @ Wk).astype(np.float32)
    bq_eff = float(wqv @ bq)
    bk_eff = float(wkv @ bk)
    g = gamma / float(N)

    import ml_dtypes
    bf = ml_dtypes.bfloat16

    wpka = np.zeros((128, 2, WPKA_COLS), np.float32)
    wpkb = np.zeros((128, 2, WPKB_COLS), np.float32)
    for q in range(2):
        cs = slice(128 * q, 128 * (q + 1))
        wpka[:, q, 0] = wq_eff[cs]
        wpka[:, q, 1] = wk_eff[cs]
        wpka[:, q, 2:130] = wq_eff[cs][:, None]
        wpkb[:, q, 0:256] = g * Wv.T[cs, :]
    wpkb[:, 0, 256:WPKB_COLS] = np.eye(128, dtype=np.float32)
    wpka = wpka.astype(bf)
    wpkb = wpkb.astype(bf)

    rpk2 = np.concatenate([bv, [g * N, 0.0]]).reshape(1, RPK_COLS).astype(bf)
    rones = np.ones((1, HALF), dtype=bf)

    xr_all = x.reshape(B, C, N)
    xb = xr_all.astype(bf).reshape(B, 2, 128, N)     # [B, q, p, n]
    in_maps = []
    for core in range(NCORES):
        b, half = core // 2, core % 2
        own = slice(HALF * half, HALF * (half + 1))
        other = slice(HALF * (1 - half), HALF * (2 - half))
        im = {
            "wpka": np.ascontiguousarray(wpka),
            "wpkb": np.ascontiguousarray(wpkb),
            "rpk2": np.ascontiguousarray(rpk2),
            "rones": np.ascontiguousarray(rones),
        }
        for s, sl in enumerate([own, other]):
            xs = xb[b][:, :, sl].transpose(1, 0, 2)
            for k in range(2):
                im[f"x{s}{k}"] = np.ascontiguousarray(
                    xs[:, :, 1024 * k:1024 * (k + 1)])
        in_maps.append(im)
    return in_maps, (g, bq_eff, bk_eff)


def assemble(results):
    """Stitch per-core halves into the full output [B, C, H, W]."""
    y = np.empty((B, C, N), dtype=np.float32)
    for core in range(NCORES):
        b, half = core // 2, core % 2
        yo = np.asarray(results[core]["yout"], dtype=np.float32)  # [128,2,2048]
        y[b, :, HALF * half:HALF * (half + 1)] = \
            yo.transpose(1, 0, 2).reshape(C, HALF)
    return y.reshape(B, C, H, W)


def kernel(**inputs):
    install_wait_split()
    in_maps, (g, bq_eff, bk_eff) = host_prep(**inputs)
    nc = build_kernel(g, bq_eff, bk_eff)
    res = run_bass_kernel_spmd(nc, in_maps, core_ids=list(range(NCORES)))
    return assemble(res.results)


# revision 42
# speedup vs baseline: 1.2478x; 1.0266x over previous
"""Trainium2 Bass kernel for Attention_concat (separable PAM attention).

Math (per batch b, N = H*W = 4096):
    eqn[n] = wq_eff . x[:, n]                  (wq_eff = Wq^T Wc[:64])
    ekn[m] = wk_eff . x[:, m]
    y[c, m] = x[c, m] + A[c] + Bv[c] * ekn[m]
with global reductions u = x @ 1, t = x @ eqn and
    Bv = g*Wv u + g*N*bv
    A  = g*Wv (t + (bq_eff+bk_eff) u) + bv*(g*E + g*N*(bq_eff+bk_eff))
    E  = wq_eff . u,   g = gamma / N

Precision: the attention correction is ~1.5e-4 of |y|, so the whole pipeline
runs in bf16 (x is loaded bf16, y stored bf16); measured rel-to-scale error
~3e-3 vs the 2e-2 gate.

Sharding: 2 cores per batch, each handles half the spatial columns. Each core
redundantly computes the global reductions over the full x[b] (own half + a
bf16 copy of the other half), then writes its own 2048 output columns.

Engine/DMA split: inputs ride both HWDGE rings (sync: x own half; scalar: the
weight pack first, then x other half) so the eqb-gating weights land early.
PE broadcasts eqn into PSUM (stationary wq_eff replicated along the free dim)
and computes ekn rows; DVE does the t-reduction via scalar_tensor_tensor with
accum_out; ACT accumulates u via activation-Copy accum_out; ekn PSUM->RC
copies split ACT/DVE. The A/Bv rows are assembled in one [2,C] PSUM
accumulation chain (tub stationary + [gN,0]/[0,sc] selector rows against the
bv row) — no cross-partition moves. Phase C: rank-2 AB x RC matmul per
512-block; two blocks finish as DVE adds (x + psum), two as PE identity-fold
plus ACT copy, then 4 output DMAs alternating rings. Dummy matmuls keep the
PE p-state up across idle windows.

Module-level workarounds (this container's walrus accepts only one sync-wait
per instruction): extra waits are hoisted onto single-wait NoOps at BIR level,
and the Tile tail drain is rebuilt the same way.
"""

import json as _json

import numpy as np

import concourse.bass as bass
import concourse.bass2jax as _b2j
import concourse.bass_utils as _bu
import concourse.mybir as mybir
import concourse.tile as tile
from concourse.bass_utils import run_bass_kernel_spmd
from concourse.tile_rust import add_dep_helper
from concourse.vector_clock import ScopedClock, VectorClock

B, C, H, W = 4, 256, 64, 64
N = H * W            # 4096
INTER = C // 4       # 64
NCORES = 8
HALF = N // 2        # 2048 output columns per core
F32 = mybir.dt.float32
BF16 = mybir.dt.bfloat16
AX = mybir.AxisListType
OP = mybir.AluOpType
ACTF = mybir.ActivationFunctionType

# wpka free-dim layout (per q chunk): [0]=wq_eff col, [1]=wk_eff col,
# [2:130]=wq_eff replicated 128  (gates phase A -> lands first)
WPKA_COLS = 130
# wpkb: [0:256]=g*Wv^T, [256:384]=identity in q=0  (tail-only -> lands last)
WPKB_COLS = 384
# rpk2 row-pack: [0:256]=bv, [256]=g*N, [257]=0
RPK_COLS = 258


def _split_multi_waits(bir: dict) -> dict:
    """The nix walrus accepts only ONE sync-wait command per instruction.
    Hoist extra waits onto preceding single-wait NoOps on the same engine
    (sequencers execute in program order, so semantics are unchanged)."""
    ctr = 0
    for fn in bir.get("functions", []):
        for blk in fn.get("blocks", []):
            insts = blk.get("instructions")
            if not insts:
                continue
            out = []
            for inst in insts:
                si = inst.get("sync_info") or {}
                waits = si.get("on_wait") or []
                if len(waits) > 1 and inst.get("engine", "Unassigned") != "Unassigned":
                    for w in waits[:-1]:
                        ctr += 1
                        out.append({
                            "debug": inst.get("debug", 0),
                            "engine": inst["engine"],
                            "ins": [], "outs": [],
                            "name": f"{inst['name']}-ws{ctr}",
                            "opcode": "NoOp",
                            "sync_info": {"on_update": [], "on_wait": [w]},
                        })
                    si["on_wait"] = [waits[-1]]
                out.append(inst)
            blk["instructions"] = out
    return bir


_WAIT_SPLIT_DONE = False


def install_wait_split():
    global _WAIT_SPLIT_DONE
    if _WAIT_SPLIT_DONE:
        return
    orig = _bu.compile_bir_kernel

    def wrapped(bir_json, *a, **kw):
        d = _json.loads(bir_json)
        _split_multi_waits(d)
        return orig(_json.dumps(d).encode(), *a, **kw)

    _bu.compile_bir_kernel = wrapped
    _b2j.compile_bir_kernel = wrapped
    _WAIT_SPLIT_DONE = True


class SplitDrainTileContext(tile.TileContext):
    """Tail fix for the same 1-wait walrus limit: park the global-clock waits
    on single-wait Nops spread across all five engines (they wait in
    parallel), then a wait-free drain + the usual barrier/reset."""

    def _drain_and_barrier(self, tick_clock, wait_clock):
        gc = tick_clock.global_clock
        nprocs = len(gc)
        engines = [self.nc.sync, self.nc.vector, self.nc.scalar,
                   self.nc.gpsimd, self.nc.tensor]
        idx = 0
        for proc in range(nprocs):
            if gc[proc] > 0:
                eng = engines[idx % len(engines)]
                idx += 1
                nop = eng.nop(nofuse=True, hint=f"tail_wait_p{proc}")
                vc = VectorClock([0] * nprocs)
                vc.require_at_least(proc, gc[proc])
                wait_clock.add_sem_waits(nop.ins, ScopedClock({None: vc}))
        self.nc.sync.drain()
        self.nc.all_engine_barrier()
        assert self.sems is not None
        popped = self.nc._tile_sem_poison_stack.pop()
        assert popped is self._sem_poison
        self.nc.clear_and_free_semaphores(list(self.sems.allocated().values()))
        self.nc.all_engine_barrier()


def build_kernel(g: float, bq_eff: float, bk_eff: float):
    """Build the per-core Bass program. g = gamma/N."""
    bqk = bq_eff + bk_eff
    nc = bass.Bass()
    xd = [[nc.dram_tensor(f"x{s}{k}", [128, 2, 1024], BF16,
                          kind="ExternalInput")
           for k in range(2)] for s in range(2)]
    wpka = nc.dram_tensor("wpka", [128, 2, WPKA_COLS], BF16, kind="ExternalInput")
    wpkb = nc.dram_tensor("wpkb", [128, 2, WPKB_COLS], BF16, kind="ExternalInput")
    rpk2 = nc.dram_tensor("rpk2", [1, RPK_COLS], BF16, kind="ExternalInput")
    rones = nc.dram_tensor("rones", [1, HALF], BF16, kind="ExternalInput")
    yout = nc.dram_tensor("yout", [128, 2, HALF], BF16, kind="ExternalOutput")

    with SplitDrainTileContext(nc) as tc:
        with (
            tc.tile_pool(name="persist", bufs=1) as pp,
            tc.tile_pool(name="trasha", bufs=1) as tpa,
            tc.tile_pool(name="trashd", bufs=1) as tpd,
            tc.tile_pool(name="ypool", bufs=4) as yp,
            tc.tile_pool(name="psm", bufs=2, space="PSUM") as psm,
            tc.tile_pool(name="pbig", bufs=2, space="PSUM") as pbig,
            tc.tile_pool(name="pwu", bufs=1, space="PSUM") as pwu,
        ):
            # --- persistent tiles -------------------------------------------
            xts = [pp.tile([128, 2, HALF], BF16, tag=f"x{s}", name=f"x{s}")
                   for s in range(2)]                    # s=0 own, s=1 other
            xt = [[xts[s][:, :, 1024 * k:1024 * (k + 1)] for k in range(2)]
                  for s in range(2)]
            wpka_sb = pp.tile([128, 2, WPKA_COLS], BF16, tag="wpka")
            wpkb_sb = pp.tile([128, 2, WPKB_COLS], BF16, tag="wpkb")
            rpk2_sb = pp.tile([1, RPK_COLS], BF16, tag="rpk2")
            RC = pp.tile([2, HALF], BF16, tag="RC")      # row0 ekn, row1 ones
            AB = pp.tile([2, C], BF16, tag="AB")         # row0 Bv, row1 A
            tacc = pp.tile([128, 2, 4], F32, tag="tacc")
            uacc = pp.tile([128, 2, 2], F32, tag="uacc")
            tu = pp.tile([128, 2, 2], F32, tag="tu")     # col0 u, col1 t+bqk*u
            tub = pp.tile([128, 2, 2], BF16, tag="tub")
            u2b = pp.tile([128, 2], BF16, tag="u2b")
            t2 = pp.tile([128, 2], F32, tag="t2")
            u2 = pp.tile([128, 2], F32, tag="u2")
            scsel = pp.tile([1, 2], BF16, tag="scsel")   # [0, sc] selector
            wusrc = pp.tile([128, 512], BF16, tag="wusrc")
            atr = pp.tile([1, 1], BF16, tag="atr")       # ACT table-load dummy

            wqcol = lambda q: wpka_sb[:, q, 0:1]
            wkcol = lambda q: wpka_sb[:, q, 1:2]
            wqrep = lambda q: wpka_sb[:, q, 2:130]
            wvt = lambda q: wpkb_sb[:, q, 0:256]
            ident = wpkb_sb[:, 0, 256:WPKB_COLS]
            bvrow = rpk2_sb[0:1, 0:C]
            cgn = rpk2_sb[0:1, C:C + 2]                  # [g*N, 0]

            # --- t=0: DMAs + cheap setup ------------------------------------
            # sync ring: the four x chunks in consumption order (FIFO per
            # ring, so each lands as the previous finishes); scalar ring:
            # weights + small rows (wpka gates the first eqb).
            nc.scalar.dma_start(out=wpka_sb, in_=wpka[:, :, :])
            for s in range(2):
                for k in range(2):
                    nc.sync.dma_start(out=xts[s][:, :, 1024 * k:1024 * (k + 1)],
                                      in_=xd[s][k][:, :, :])
            nc.scalar.dma_start(out=wpkb_sb, in_=wpkb[:, :, :])
            nc.scalar.dma_start(out=rpk2_sb, in_=rpk2[:, :])
            nc.scalar.dma_start(out=RC[1:2, :], in_=rones[:, :])

            nc.vector.memset(wusrc, 0.5)
            nc.vector.memset(scsel, 0.0)
            # ACT function-table load happens at the first activation: trigger
            # it early on a 1-element dummy so it overlaps the DMA wait.
            nc.scalar.activation(out=atr, in_=wusrc[0:1, 0:1], func=ACTF.Copy)

            # PE p-state ramp: dummy matmuls with no DMA dependency.
            def dummy_mm(n, tag):
                for i in range(n):
                    wu = pwu.tile([128, 512], F32, tag="wu", name=f"wu_{tag}_{i}")
                    nc.tensor.matmul(wu, wusrc[:, 0:128], wusrc,
                                     start=True, stop=True)

            dummy_mm(4, "pre")

            # --- phase A: stream x, eq broadcast, t/u reductions, ekn -------
            for sb in range(4):
                s, k = sb // 2, sb % 2
                src = xt[s][k]
                # eq broadcast: [128, 1024] PSUM, 2 blocks x 2 q-chunks
                eqb = pbig.tile([128, 1024], F32, tag="big", name=f"eqb{sb}")
                for half in range(2):
                    blk = slice(512 * half, 512 * (half + 1))
                    for q in range(2):
                        nc.tensor.matmul(eqb[:, blk], wqrep(q), src[:, q, blk],
                                         start=(q == 0), stop=(q == 1))
                dummy_mm(1, f"a{sb}")
                # u accumulation on ACT: one [128, 2048] pass per (s, q)
                if k == 1:
                    for q in range(2):
                        trsh = tpa.tile([128, 2048], BF16, tag="tr")
                        last_u = nc.scalar.activation(
                            out=trsh, in_=xts[s][:, q, :], func=ACTF.Copy,
                            accum_out=uacc[:, q, s:s + 1])
                # t reduction: fused (eqb+0)*x with free-dim accumulate (DVE)
                for q in range(2):
                    trsh = tpd.tile([128, 1024], BF16, tag="tr")
                    last_stt = nc.vector.scalar_tensor_tensor(
                        out=trsh, in0=eqb, scalar=0.0, in1=src[:, q, :],
                        op0=OP.add, op1=OP.mult,
                        accum_out=tacc[:, q, sb:sb + 1])

            # --- tail: fold reductions into the AB rows ---------------------
            # ekn rows now: the matmuls keep the PE p-state up through the
            # fold window, and ACT (idle during folds) does the RC copies.
            dummy_mm(14, "t0")
            nek = 0
            for k in range(2):
                for half in range(2):
                    blk = slice(512 * half, 512 * (half + 1))
                    gcol = slice(1024 * k + 512 * half,
                                 1024 * k + 512 * half + 512)
                    ekp = psm.tile([1, 512], F32, tag="sm",
                                   name=f"ek{k}_{half}")
                    for q in range(2):
                        nc.tensor.matmul(ekp, wkcol(q), xt[0][k][:, q, blk],
                                         start=(q == 0), stop=(q == 1))
                    if nek < 3:
                        cp = nc.scalar.copy(out=RC[0:1, gcol], in_=ekp)
                        add_dep_helper(cp.ins, last_u.ins, sync=False,
                                       reason="ek copies after u stream")
                    else:
                        cp = nc.vector.tensor_copy(out=RC[0:1, gcol], in_=ekp)
                        add_dep_helper(cp.ins, last_stt.ins, sync=False,
                                       reason="ek copy after stt stream")
                    nek += 1
            nc.vector.tensor_reduce(out=u2, in_=uacc, axis=AX.X, op=OP.add)
            nc.vector.tensor_copy(out=u2b, in_=u2)
            nc.vector.tensor_reduce(out=t2, in_=tacc, axis=AX.X, op=OP.add)
            nc.vector.tensor_copy(out=tu[:, :, 0], in_=u2)
            nc.vector.tensor_scalar(out=tu[:, :, 1], in0=u2,
                                    scalar1=bqk, scalar2=None, op0=OP.mult)
            nc.vector.tensor_tensor(out=tu[:, :, 1], in0=tu[:, :, 1],
                                    in1=t2, op=OP.add)
            nc.vector.tensor_copy(out=tub, in_=tu)

            # E = wq_eff . u -> sc = g*E + g*N*bqk into scsel = [0, sc]
            # (runs off u2b so it overlaps the t folds above)
            ep = psm.tile([1, 1], F32, tag="sm", name="ep")
            for q in range(2):
                nc.tensor.matmul(ep, u2b[:, q:q + 1], wqcol(q),
                                 start=(q == 0), stop=(q == 1))
            nc.scalar.activation(out=scsel[0:1, 1:2], in_=ep, func=ACTF.Copy,
                                 scale=g, bias=g * N * bqk)
            # AB rows in one [2, C] PSUM accumulation chain:
            #   row0 (Bv) = g*Wv u        + g*N*bv + 0*bv
            #   row1 (A)  = g*Wv(t+bqk u) + 0      + sc*bv
            P = psm.tile([2, C], F32, tag="sm", name="P")
            for q in range(2):
                nc.tensor.matmul(P, tub[:, q, :], wvt(q),
                                 start=(q == 0), stop=False)
            nc.tensor.matmul(P, cgn, bvrow, start=False, stop=False)
            nc.tensor.matmul(P, scsel, bvrow, start=False, stop=True)
            dummy_mm(2, "t1")
            nc.vector.tensor_copy(out=AB, in_=P)

            # --- phase C: y = x + A + Bv*ekn over own half ------------------
            # First two blocks (k=0): PE identity-fold opens the PSUM group
            # early (x-only dependency), rank-2 closes it after AB, ACT does
            # the move-out. Last two blocks (k=1): rank-2 only + DVE x-add.
            dma_eng = [nc.sync, nc.scalar, nc.sync, nc.scalar]
            bi = 0
            for k in range(2):
                for q in range(2):
                    on_dve = (k == 1)
                    yps = pbig.tile([128, 1024], F32, tag="big",
                                    name=f"yps{q}_{k}")
                    for half in range(2):
                        blk = slice(512 * half, 512 * (half + 1))
                        if not on_dve:
                            nc.tensor.matmul(yps[:, blk], ident,
                                             xt[0][k][:, q, blk],
                                             start=True, stop=False)
                    for half in range(2):
                        blk = slice(512 * half, 512 * (half + 1))
                        gcol = slice(1024 * k + 512 * half,
                                     1024 * k + 512 * half + 512)
                        nc.tensor.matmul(yps[:, blk],
                                         AB[:, 128 * q:128 * (q + 1)],
                                         RC[0:2, gcol], start=on_dve,
                                         stop=True)
                    ysb = yp.tile([128, 1024], BF16, tag="y")
                    if on_dve:
                        nc.vector.tensor_tensor(out=ysb, in0=xt[0][k][:, q, :],
                                                in1=yps, op=OP.add)
                    else:
                        nc.scalar.activation(out=ysb, in_=yps, func=ACTF.Copy)
                    dma_eng[bi].dma_start(
                        out=yout[:, q, 1024 * k:1024 * (k + 1)], in_=ysb)
                    bi += 1
    return nc


def host_prep(x, Wq, bq, Wk, bk, Wc, Wv, bv, gamma):
    """Fold weights on host; build per-core input maps."""
    x = np.asarray(x, dtype=np.float32)
    Wq = np.asarray(Wq, np.float32); bq = np.asarray(bq, np.float32)
    Wk = np.asarray(Wk, np.float32); bk = np.asarray(bk, np.float32)
    Wc = np.asarray(Wc, np.float32)
    Wv = np.asarray(Wv, np.float32); bv = np.asarray(bv, np.float32)
    gamma = float(np.asarray(gamma).reshape(-1)[0])

    wqv, wkv = Wc[:INTER], Wc[INTER:]
    wq_eff = (wqv @ Wq).astype(np.float32)          # [C]
    wk_eff = (wkv @ Wk).astype(np.float32)
    bq_eff = float(wqv @ bq)
    bk_eff = float(wkv @ bk)
    g = gamma / float(N)

    import ml_dtypes
    bf = ml_dtypes.bfloat16

    wpka = np.zeros((128, 2, WPKA_COLS), np.float32)
    wpkb = np.zeros((128, 2, WPKB_COLS), np.float32)
    for q in range(2):
        cs = slice(128 * q, 128 * (q + 1))
        wpka[:, q, 0] = wq_eff[cs]
        wpka[:, q, 1] = wk_eff[cs]
        wpka[:, q, 2:130] = wq_eff[cs][:, None]
        wpkb[:, q, 0:256] = g * Wv.T[cs, :]
    wpkb[:, 0, 256:WPKB_COLS] = np.eye(128, dtype=np.float32)
    wpka = wpka.astype(bf)
    wpkb = wpkb.astype(bf)

    rpk2 = np.concatenate([bv, [g * N, 0.0]]).reshape(1, RPK_COLS).astype(bf)
    rones = np.ones((1, HALF), dtype=bf)

    xr_all = x.reshape(B, C, N)
    xb = xr_all.astype(bf).reshape(B, 2, 128, N)     # [B, q, p, n]
    in_maps = []
    for core in range(NCORES):
        b, half = core // 2, core % 2
        own = slice(HALF * half, HALF * (half + 1))
        other = slice(HALF * (1 - half), HALF * (2 - half))
        im = {
            "wpka": np.ascontiguousarray(wpka),
            "wpkb": np.ascontiguousarray(wpkb),
            "rpk2": np.ascontiguousarray(rpk2),
            "rones": np.ascontiguousarray(rones),
        }
        for s, sl in enumerate([own, other]):
            xs = xb[b][:, :, sl].transpose(1, 0, 2)
            for k in range(2):
                im[f"x{s}{k}"] = np.ascontiguousarray(
                    xs[:, :, 1024 * k:1024 * (k + 1)])
        in_maps.append(im)
    return in_maps, (g, bq_eff, bk_eff)


def assemble(results):
    """Stitch per-core halves into the full output [B, C, H, W]."""
    y = np.empty((B, C, N), dtype=np.float32)
    for core in range(NCORES):
        b, half = core // 2, core % 2
        yo = np.asarray(results[core]["yout"], dtype=np.float32)  # [128,2,2048]
        y[b, :, HALF * half:HALF * (half + 1)] = \
            yo.transpose(1, 0, 2).reshape(C, HALF)
    return y.reshape(B, C, H, W)


def kernel(**inputs):
    install_wait_split()
    in_maps, (g, bq_eff, bk_eff) = host_prep(**inputs)
    nc = build_kernel(g, bq_eff, bk_eff)
    res = run_bass_kernel_spmd(nc, in_maps, core_ids=list(range(NCORES)))
    return assemble(res.results)
